# revision 15
# baseline (speedup 1.0000x reference)
"""3-layer GCN (PyG GCNConv x3, relu between) on 8 Trainium2 NeuronCores.

Math: out = A*(relu(A*(relu(A*(xW1)+b1)W2+b2))W3)+b3 with A = D^-1/2(A+I)D^-1/2.
htilde = dinv * (input @ W) per layer (dense, PE, bf16), AllGather htilde
(bf16, 128-wide rows) across the 8 cores, then aggregation is:
dma_gather of the DENSE per-edge message stream (no interior K-padding;
~213k idxs/core/layer vs 523k in the padded scheme - dma_gather costs
7.9ns/idx of GpSimd descriptor generation regardless of row width, so idx
count is everything), followed by TensorEngine one-hot routing: each
128-message gather column is matmul'd with a host-precomputed S matrix
(S[m, lane] = dinv[dst_m], else 0) accumulating into a per-block PSUM tile,
finalized into an SBUF accumulator A. Pad idxs point at row 0 (real, finite
data) and their S rows are zero. Message stream is sorted (src-quarter,
dst-block) with per-(q,block) capacity = max over cores (64-aligned), so one
SPMD program serves all 8 cores; per-core data (idxs, S) carries the graph.
"""
import sys, time
sys.path.insert(0, "/opt/trn_rl_repo")
import numpy as np
import ml_dtypes

N = 100_000
DIMS = [512, 128, 64, 32]
NCORES = 8
P = 128
SLOTS = 12544          # 98 blocks * 128 lanes per core
NBLK = SLOTS // P      # 98
HROWS = NCORES * SLOTS # 100352
NQ = 4
QROWS = HROWS // NQ    # 25088 (< 32768, int16-addressable)
CALL = 3072            # idxs per dma_gather call

_CACHE = {}


# --------------------------------------------------------------------------
# host-side graph preprocessing
# --------------------------------------------------------------------------
def _preprocess(edge_index):
    src = np.asarray(edge_index[0], np.int64)
    dst = np.asarray(edge_index[1], np.int64)
    deg = np.bincount(dst, minlength=N).astype(np.int64) + 1  # + self loop
    dinv = (1.0 / np.sqrt(deg)).astype(np.float32)

    rank = np.argsort(-deg, kind="stable")
    pos = np.empty(N, np.int64); pos[rank] = np.arange(N)
    core_of = pos % NCORES
    slot_of = pos // NCORES

    S_ = np.concatenate([src, np.arange(N)])
    D_ = np.concatenate([dst, np.arange(N)])
    ec = core_of[D_]
    eslot = slot_of[D_]
    eblk, elane = eslot // P, eslot % P
    hrow = core_of[S_] * SLOTS + slot_of[S_]
    eq, eqidx = hrow // QROWS, hrow % QROWS

    # per-(core, quarter, block) counts -> uniform 64-aligned capacities
    cnt = np.zeros((NCORES, NQ, NBLK), np.int64)
    np.add.at(cnt, (ec, eq, eblk), 1)
    cap = ((cnt.max(axis=0) + 15) // 16) * 16        # [NQ, NBLK]
    # align each quarter's total to a multiple of 128 so every gather call
    # covers whole 128-msg columns
    for q in range(NQ):
        cap[q, NBLK - 1] += (-cap[q].sum()) % 128
    qtot = cap.sum(axis=1)                            # per-quarter stream len
    qoff = np.concatenate([[0], np.cumsum(qtot)])[:NQ]
    TI = int(cap.sum())                               # total stream length

    # group offsets in the stream (quarter-major, block order inside)
    goff = np.zeros((NQ, NBLK), np.int64)
    for q in range(NQ):
        goff[q] = qoff[q] + np.concatenate([[0], np.cumsum(cap[q])[:-1]])

    # ops: one matmul per (group, 128-col piece); uniform across cores
    # op list entries: (q, j, col, r0, r1, start, stop)
    ops = []
    op_base = np.zeros((NQ, NBLK), np.int64)
    first_col = np.zeros((NQ, NBLK), np.int64)
    for q in range(NQ):
        for j in range(NBLK):
            a = goff[q, j]; b = a + cap[q, j]
            c0, c1 = a // 128, (b - 1) // 128
            op_base[q, j] = len(ops)
            first_col[q, j] = c0
            for c in range(c0, c1 + 1):
                r0 = max(a, c * 128) - c * 128
                r1 = min(b, (c + 1) * 128) - c * 128
                ops.append((q, j, c, r0, r1, c == c0, c == c1))
    NS = len(ops)

    # calls: chunks of <=CALL idxs within each quarter
    calls = []   # (q, pos0, n, col0)
    for q in range(NQ):
        p0 = qoff[q]
        qend = qoff[q] + qtot[q]
        while p0 < qend:
            n = min(CALL, qend - p0)
            calls.append((q, int(p0), int(n), int(p0 // 128)))
            p0 += n
    # op -> call mapping (by column)
    col2call = {}
    for ci, (q, p0, n, c0) in enumerate(calls):
        for c in range(c0, (p0 + n) // 128):
            col2call[c] = ci
    # S tiles are batched per call: s column offset of op within its call
    ops_by_call = [[] for _ in calls]
    for k, (q, j, c, r0, r1, st, sp) in enumerate(ops):
        ops_by_call[col2call[c]].append(k)
    s_off = np.zeros(NS, np.int64)   # tile index within call batch
    s_cnt = [len(v) for v in ops_by_call]
    for v in ops_by_call:
        for t, k in enumerate(v):
            s_off[k] = t

    # per-edge stream positions
    order = np.lexsort((eblk, eq, ec))
    oc, oq, ob = ec[order], eq[order], eblk[order]
    grp = (oc * NQ + oq) * NBLK + ob
    newgrp = np.r_[True, grp[1:] != grp[:-1]]
    first = np.flatnonzero(newgrp)
    within = np.arange(len(grp)) - first[np.cumsum(newgrp) - 1]
    gpos = goff[oq, ob] + within
    assert (within < cap[oq, ob]).all()

    gidx_full = np.zeros((NCORES, TI), np.int64)      # pad idx = 0 (real row)
    gidx_full[oc, gpos] = eqidx[order]
    ecol = gpos // 128
    erow = gpos % 128
    eop = op_base[oq, ob] + (ecol - first_col[oq, ob])

    # S values: dinv[dst] at (row, lane); bf16, batched per call
    sval = np.zeros((NCORES, P, NS * P), ml_dtypes.bfloat16)
    sval[oc, erow, eop * P + elane[order]] = dinv[D_[order]]

    # wrapped idx layout per call: [16, n/16] tiled to 128 partitions
    wrapped = np.empty((NCORES, P, TI // 16), np.int16)
    for c in range(NCORES):
        col = 0
        for (q, p0, n, c0) in calls:
            w = gidx_full[c, p0:p0 + n].reshape(-1, 16).T.astype(np.int16)
            wrapped[c, :, col:col + n // 16] = np.tile(w, (8, 1))
            col += n // 16

    ids = np.full((NCORES, SLOTS), -1, np.int64)
    ids[core_of, slot_of] = np.arange(N)
    dinv_t = np.zeros((NCORES, P, NBLK), np.float32)
    for c in range(NCORES):
        v = ids[c]
        dv = np.where(v >= 0, dinv[np.maximum(v, 0)], 0.0).astype(np.float32)
        dinv_t[c] = dv.reshape(NBLK, P).T
    return dict(ids=ids, ops=ops, calls=calls, ops_by_call=ops_by_call,
                s_off=s_off, s_cnt=s_cnt, NS=NS, TI=TI,
                wrapped=wrapped, sval=sval, dinv_t=dinv_t)


# --------------------------------------------------------------------------
# bass program
# --------------------------------------------------------------------------
def _build(pre):
    from concourse import bass, bacc, mybir, tile
    from concourse.library_config import mlp
    from concourse.masks import make_identity
    AL = mybir.AluOpType
    f32, i16, bf16 = mybir.dt.float32, mybir.dt.int16, mybir.dt.bfloat16
    ops, calls = pre["ops"], pre["calls"]
    ops_by_call, s_off, s_cnt = pre["ops_by_call"], pre["s_off"], pre["s_cnt"]
    NS, TI = pre["NS"], pre["TI"]

    nc = bacc.Bacc("TRN2", target_bir_lowering=False, debug=False,
                   num_devices=NCORES, num_swdge_queues=4)
    xT_in = nc.dram_tensor("xT", (DIMS[0], SLOTS), bf16, kind="ExternalInput")
    idx_in = nc.dram_tensor("gidx", (P, TI // 16), i16, kind="ExternalInput")
    sv_in = nc.dram_tensor("sval", (P, NS * P), bf16, kind="ExternalInput")
    dinv_in = nc.dram_tensor("dinv", (P, NBLK), f32, kind="ExternalInput")
    w_in = [nc.dram_tensor(f"W{i+1}", (DIMS[i], DIMS[i + 1]), bf16, kind="ExternalInput") for i in range(3)]
    b_in = [nc.dram_tensor(f"b{i+1}", (P, DIMS[i + 1]), f32, kind="ExternalInput") for i in range(3)]
    out_t = nc.dram_tensor("out", (SLOTS, DIMS[3]), f32, kind="ExternalOutput")

    with tile.TileContext(nc) as tc:
        with tc.tile_pool(name="const", bufs=1) as const, \
             tc.tile_pool(name="gath", bufs=8) as gpool, \
             tc.tile_pool(name="sp", bufs=4) as spool, \
             tc.tile_pool(name="work", bufs=3) as work, \
             tc.tile_pool(name="lhs", bufs=8) as lhs, \
             tc.tile_pool(name="pagg", bufs=2, space="PSUM") as pagg, \
             tc.tile_pool(name="pps", bufs=2, space="PSUM") as pps, \
             tc.tile_pool(name="ppt", bufs=2, space="PSUM") as ppt, \
             tc.tile_pool(name="dram", bufs=1, space="DRAM") as dram:

            nc.gpsimd.load_library(mlp)
            idx_t = const.tile([P, TI // 16], i16, tag="idx")
            nc.sync.dma_start(out=idx_t[:], in_=idx_in[:, :])
            dinv_t = const.tile([P, NBLK], f32, tag="dinv")
            nc.sync.dma_start(out=dinv_t[:], in_=dinv_in[:, :])
            ident = const.tile([P, P], f32, tag="ident")
            make_identity(nc, ident[:])
            ztb = const.tile([P, P], bf16, tag="zerob")
            nc.vector.memset(ztb[:], 0.0)
            w1t = [const.tile([P, DIMS[1]], bf16, tag=f"w1_{k}", name=f"w1_{k}") for k in range(4)]
            for k in range(4):
                nc.sync.dma_start(out=w1t[k][:], in_=w_in[0][k * P:(k + 1) * P, :])
            w2t = const.tile([DIMS[1], DIMS[2]], bf16, tag="w2")
            nc.sync.dma_start(out=w2t[:], in_=w_in[1][:, :])
            w3t = const.tile([DIMS[2], DIMS[3]], bf16, tag="w3")
            nc.sync.dma_start(out=w3t[:], in_=w_in[2][:, :])
            bt = []
            for i in range(3):
                t = const.tile([P, DIMS[i + 1]], f32, tag=f"b{i}", name=f"bt{i}")
                nc.sync.dma_start(out=t[:], in_=b_in[i][:, :])
                bt.append(t)
            acc = const.tile([P, NBLK, P], f32, tag="acc")  # A accumulator

            slab = dram.tile([SLOTS, P], bf16, tag="slab", name="slab")
            hbufs = [dram.tile([HROWS, P], bf16, tag=f"hbuf{i}",
                               name=f"hbuf{i}", addr_space="Shared")
                     for i in range(3)]

            def agg_layer(dreal, lidx, finalize_cb=None):
                hbuf = hbufs[lidx]
                nc.gpsimd.collective_compute(
                    "AllGather", AL.bypass,
                    replica_groups=[list(range(NCORES))],
                    ins=[slab.opt()], outs=[hbuf.opt()])
                idx_col = 0
                s_base = 0
                chain_ps = None
                chain_j = -1
                for ci, (q, p0, n, c0) in enumerate(calls):
                    gt = gpool.tile([P, n // P, P], bf16, tag="gt")
                    nc.gpsimd.dma_gather(
                        out_ap=gt[:],
                        in_ap=hbuf[q * QROWS:(q + 1) * QROWS, :],
                        idxs_ap=idx_t[:, idx_col:idx_col + n // 16],
                        num_idxs=n, num_idxs_reg=n, elem_size=P,
                        single_packet=False, queue_num=ci % 4)
                    idx_col += n // 16
                    ns = s_cnt[ci]
                    st = spool.tile([P, ns * P], bf16, tag="st")
                    nc.sync.dma_start(
                        out=st[:], in_=sv_in[:, s_base * P:(s_base + ns) * P])
                    for k in ops_by_call[ci]:
                        q_, j, c, r0, r1, is_start, is_stop = ops[k]
                        if is_start:
                            chain_ps = pagg.tile([P, P], f32, space="PSUM",
                                                 tag="cps")
                            chain_j = j
                        nc.tensor.matmul(
                            out=chain_ps[:, :dreal],
                            lhsT=st[:, s_off[k] * P:(s_off[k] + 1) * P],
                            rhs=gt[:, c - c0, :dreal],
                            start=is_start, stop=is_stop)
                        if is_stop:
                            if q_ == 0 and NQ > 1:
                                nc.vector.tensor_copy(
                                    out=acc[:, j, :dreal],
                                    in_=chain_ps[:, :dreal])
                            else:
                                nc.vector.tensor_tensor(
                                    out=acc[:, j, :dreal],
                                    in0=acc[:, j, :dreal],
                                    in1=chain_ps[:, :dreal], op=AL.add)
                            if q_ == NQ - 1 and finalize_cb is not None:
                                finalize_cb(j)
                    s_base += ns

            def dscale(dst_ap, src_ap, j, d):
                nc.vector.tensor_tensor(
                    out=dst_ap, in0=src_ap,
                    in1=dinv_t[:, j:j + 1].to_broadcast([P, d]), op=AL.mult)

            # ---- L1 transform: htilde1 = dinv * (x @ W1), slab rows bf16
            # xT loaded in 8-block batches to amortize DMA issue cost
            JG = 8
            for jg in range(0, NBLK, JG):
                nb = min(JG, NBLK - jg)
                lts = []
                for k in range(4):
                    lt = lhs.tile([P, JG * P], bf16, tag="xT")
                    nc.sync.dma_start(out=lt[:, :nb * P],
                                      in_=xT_in[k * P:(k + 1) * P, jg * P:(jg + nb) * P])
                    lts.append(lt)
                for t in range(nb):
                    j = jg + t
                    ps = pps.tile([P, P], f32, space="PSUM", tag="tps")
                    for k in range(4):
                        nc.tensor.matmul(out=ps[:, :DIMS[1]],
                                         lhsT=lts[k][:, t * P:(t + 1) * P],
                                         rhs=w1t[k][:],
                                         start=(k == 0), stop=(k == 3))
                    ht = work.tile([P, DIMS[1]], bf16, tag="hrow")
                    dscale(ht[:], ps[:, :DIMS[1]], j, DIMS[1])
                    nc.sync.dma_start(out=slab[j * P:(j + 1) * P, :], in_=ht[:])

            # ---- L1 aggregate + L2 transform (interleaved per block)
            def l2_transform(j):
                s = work.tile([P, DIMS[1]], f32, tag="s1")
                nc.vector.tensor_tensor(out=s[:], in0=acc[:, j, :DIMS[1]],
                                        in1=bt[0][:], op=AL.add)
                nc.vector.tensor_scalar_max(out=s[:], in0=s[:], scalar1=0.0)
                pt = ppt.tile([P, P], f32, space="PSUM", tag="trp")
                nc.tensor.transpose(out=pt[:DIMS[1], :], in_=s[:], identity=ident[:])
                sT = work.tile([DIMS[1], P], bf16, tag="sT")
                nc.vector.tensor_copy(out=sT[:], in_=pt[:DIMS[1], :])
                ps = pps.tile([P, P], f32, space="PSUM", tag="tps")
                nc.tensor.matmul(out=ps[:, :DIMS[2]], lhsT=sT[:], rhs=w2t[:], start=True, stop=True)
                ht = work.tile([P, DIMS[2]], bf16, tag="h2row")
                dscale(ht[:], ps[:, :DIMS[2]], j, DIMS[2])
                nc.sync.dma_start(out=slab[j * P:(j + 1) * P, :DIMS[2]], in_=ht[:])
                nc.sync.dma_start(out=slab[j * P:(j + 1) * P, DIMS[2]:], in_=ztb[:, :P - DIMS[2]])
            agg_layer(DIMS[1], 0, l2_transform)

            # ---- L2 aggregate + L3 transform (interleaved per block)
            def l3_transform(j):
                s = work.tile([P, DIMS[2]], f32, tag="s2")
                nc.vector.tensor_tensor(out=s[:], in0=acc[:, j, :DIMS[2]],
                                        in1=bt[1][:], op=AL.add)
                nc.vector.tensor_scalar_max(out=s[:], in0=s[:], scalar1=0.0)
                pt = ppt.tile([P, P], f32, space="PSUM", tag="trp")
                nc.tensor.transpose(out=pt[:DIMS[2], :], in_=s[:], identity=ident[:])
                sT = work.tile([DIMS[2], P], bf16, tag="s3T")
                nc.vector.tensor_copy(out=sT[:], in_=pt[:DIMS[2], :])
                ps = pps.tile([P, P], f32, space="PSUM", tag="tps")
                nc.tensor.matmul(out=ps[:, :DIMS[3]], lhsT=sT[:], rhs=w3t[:], start=True, stop=True)
                ht = work.tile([P, DIMS[3]], bf16, tag="h3row")
                dscale(ht[:], ps[:, :DIMS[3]], j, DIMS[3])
                nc.sync.dma_start(out=slab[j * P:(j + 1) * P, :DIMS[3]], in_=ht[:])
                nc.sync.dma_start(out=slab[j * P:(j + 1) * P, DIMS[3]:], in_=ztb[:, :P - DIMS[3]])
            agg_layer(DIMS[2], 1, l3_transform)

            # ---- L3 aggregate -> output (bias only, f32 out)
            def out_write(j):
                s = work.tile([P, DIMS[3]], f32, tag="s3")
                nc.vector.tensor_tensor(out=s[:], in0=acc[:, j, :DIMS[3]],
                                        in1=bt[2][:], op=AL.add)
                nc.sync.dma_start(out=out_t[j * P:(j + 1) * P, :], in_=s[:])
            agg_layer(DIMS[3], 2, out_write)
    nc.compile()
    return nc


# --------------------------------------------------------------------------
# SPMD runner (shard_map over 8 axon cores, reusable jitted executable)
# --------------------------------------------------------------------------
class _Runner:
    def __init__(self, nc, n_cores=NCORES):
        import jax
        from jax.sharding import Mesh, PartitionSpec
        from jax.experimental.shard_map import shard_map
        from concourse import bass2jax, mybir
        bass2jax.install_neuronx_cc_hook()
        self.jax = jax
        self.nc = nc
        self.n_cores = n_cores
        pname = nc.partition_id_tensor.name if nc.partition_id_tensor else None
        in_names, out_names, out_avals, zero_outs = [], [], [], []
        for alloc in nc.m.functions[0].allocations:
            if not isinstance(alloc, mybir.MemoryLocationSet):
                continue
            name = alloc.memorylocations[0].name
            if alloc.kind == "ExternalInput":
                if name != pname:
                    in_names.append(name)
            elif alloc.kind == "ExternalOutput":
                out_names.append(name)
                out_avals.append(jax.core.ShapedArray(tuple(alloc.tensor_shape), mybir.dt.np(alloc.dtype)))
                zero_outs.append(np.zeros(tuple(alloc.tensor_shape), mybir.dt.np(alloc.dtype)))
        self.in_names, self.out_names = in_names, out_names
        self.out_avals, self.zero_outs = out_avals, zero_outs
        n_params, n_outs = len(in_names), len(out_names)
        all_in = in_names + out_names + ([pname] if pname else [])

        def _body(*args):
            operands = list(args)
            if pname:
                operands.append(bass2jax.partition_id_tensor())
            outs = bass2jax._bass_exec_p.bind(
                *operands, out_avals=tuple(out_avals), in_names=tuple(all_in),
                out_names=tuple(out_names), lowering_input_output_aliases=(),
                sim_require_finite=True, sim_require_nnan=True, nc=nc)
            return tuple(outs)

        devices = jax.devices()[:n_cores]
        self.mesh = Mesh(np.asarray(devices), ("core",))
        self.pspec = PartitionSpec("core")
        self.fn = jax.jit(
            shard_map(_body, mesh=self.mesh,
                      in_specs=(self.pspec,) * (n_params + n_outs),
                      out_specs=(self.pspec,) * n_outs, check_rep=False),
            donate_argnums=tuple(range(n_params, n_params + n_outs)),
            keep_unused=True)

    def place(self, in_maps):
        sh = self.jax.sharding.NamedSharding(self.mesh, self.pspec)
        return [self.jax.device_put(
                    np.concatenate([np.asarray(in_maps[c][n]) for c in range(self.n_cores)], axis=0), sh)
                for n in self.in_names]

    def make_zeros(self):
        sh = self.jax.sharding.NamedSharding(self.mesh, self.pspec)
        zeros = [self.jax.device_put(
                    np.zeros((self.n_cores * z.shape[0], *z.shape[1:]), z.dtype), sh)
                 for z in self.zero_outs]
        self.jax.block_until_ready(zeros)
        return zeros

    def run(self, args, zeros=None):
        if zeros is None:
            zeros = self.make_zeros()
        outs = self.fn(*args, *zeros)
        self.jax.block_until_ready(outs)
        return outs

    def results(self, outs):
        return [{n: np.asarray(outs[i]).reshape(self.n_cores, *self.out_avals[i].shape)[c]
                 for i, n in enumerate(self.out_names)}
                for c in range(self.n_cores)]


# --------------------------------------------------------------------------
def _make_in_maps(pre, np_inputs):
    x = np.asarray(np_inputs["x"], np.float32)
    ids = pre["ids"]
    in_maps = []
    for c in range(NCORES):
        v = ids[c]
        xc = np.zeros((SLOTS, DIMS[0]), np.float32)
        m = v >= 0
        xc[m] = x[v[m]]
        in_maps.append({
            "xT": np.ascontiguousarray(xc.T).astype(ml_dtypes.bfloat16),
            "gidx": pre["wrapped"][c],
            "sval": pre["sval"][c],
            "dinv": pre["dinv_t"][c],
            "W1": np.asarray(np_inputs["W1"], np.float32).astype(ml_dtypes.bfloat16),
            "W2": np.asarray(np_inputs["W2"], np.float32).astype(ml_dtypes.bfloat16),
            "W3": np.asarray(np_inputs["W3"], np.float32).astype(ml_dtypes.bfloat16),
            "b1": np.tile(np.asarray(np_inputs["b1"], np.float32)[None, :], (P, 1)),
            "b2": np.tile(np.asarray(np_inputs["b2"], np.float32)[None, :], (P, 1)),
            "b3": np.tile(np.asarray(np_inputs["b3"], np.float32)[None, :], (P, 1)),
        })
    return in_maps


def _get(edge_index):
    key = hash(np.asarray(edge_index)[:, ::997].tobytes())
    if key not in _CACHE:
        pre = _preprocess(edge_index)
        nc = _build(pre)
        _CACHE[key] = (pre, _Runner(nc))
    return _CACHE[key]


def kernel(x, edge_index, W1, b1, W2, b2, W3, b3):
    pre, runner = _get(edge_index)
    in_maps = _make_in_maps(pre, {"x": x, "W1": W1, "b1": b1, "W2": W2,
                                  "b2": b2, "W3": W3, "b3": b3})
    args = runner.place(in_maps)
    outs = runner.run(args)
    res = runner.results(outs)
    full = np.zeros((N, DIMS[3]), np.float32)
    ids = pre["ids"]
    for c in range(NCORES):
        v = ids[c]
        m = v >= 0
        full[v[m]] = res[c]["out"][m]
    return full
